# revision 18
# baseline (speedup 1.0000x reference)
"""Multi-head attention (B=2, S=2048, D=1024, H=16) on 8 TRN2 NeuronCores.

Sharding: data-parallel over batch (2) x tensor-parallel over head groups (4).
Core c handles batch c//4, heads [4*(c%4), 4*(c%4)+4).  Each core computes its
heads' attention plus its slice of the output projection (Wo row-slice); the
final all-reduce over head groups happens during the host-side gather-sum.

Per-core device pipeline (matmul operands fp16, accumulation fp32):
  x^T staged in DRAM by the host -> Q^T/K^T ([128,2048] head-pair packed, head
  dims on partitions) and V ([2048, 4x65] with a ones column per head).
  scores^T = K @ Q^T per head via two row-tiled K=64 matmuls running
  concurrently on disjoint PE row groups -> exp on ScalarE (scale=1/8,
  bias=-12 for fp16 range) -> causal diagonal-block mask multiply on GpSimd
  -> PV with V_aug stationary, whose ones column also accumulates the softmax
  denominators -> normalize (denom*256, PE ones-broadcast,
  reciprocal_approx_fast) -> out^T.T @ Wo.
Causal structure is exploited at 128-column granularity: diagonal-band tiles
compute only their valid query range (the narrow tiles are issued first in
each accumulation group; a full-width tile closes it).  k-side bias is
softmax-invariant and dropped; the v-side bias is folded into the host-side
output bias (softmax weights sum to one).  Projection matmul chains are
interleaved between attention iterations to keep the PE dense.
"""

import functools

import numpy as np

import concourse.bass as bass
import concourse.mybir as mybir
from concourse import bacc
from concourse.tile import TileContext
from concourse.bass_utils import run_bass_kernel_spmd

P = 128
S = 2048
D = 1024
H = 16
HD = 64
B = 2
NCORES = 8
HGROUPS = 4
HC = H // HGROUPS          # 4 heads per core
DC = HC * HD               # 256-wide weight slice per core
NST = S // P               # 16 s-tiles (= k-tiles inside attention)
NKT = D // P               # 8 contraction tiles for the projections
QCW = 512
NQC = S // QCW             # 4 q-chunks
VW = HD + 1                # V block width incl. ones column

F32 = mybir.dt.float32
F16 = mybir.dt.float16
AF = mybir.ActivationFunctionType
OP = mybir.AluOpType
EXP_BIAS = -12.0           # keeps exp(q.k/8 - 12) inside fp16 range


def _build(mode):
    """mode: 'causal' | 'allones' | 'general'."""
    nc = bacc.Bacc("TRN2", debug=False, num_devices=NCORES,
                   num_swdge_queues=4)

    # All inputs are pre-arranged by the host into per-partition-contiguous
    # layouts so every load is 128 large descriptors (4-8KB) instead of
    # thousands of 512B gathers -- the input DMA is descriptor-bound.
    xt_in = nc.dram_tensor("xt", [P, NQC, NKT, QCW], F16,
                           kind="ExternalInput")
    wq = nc.dram_tensor("wq", [P, NKT, DC], F16, kind="ExternalInput")
    wk = nc.dram_tensor("wk", [P, NKT, DC], F16, kind="ExternalInput")
    wv = nc.dram_tensor("wv", [P, NKT, DC], F16, kind="ExternalInput")
    wo = nc.dram_tensor("wo", [P, 2, D], F16, kind="ExternalInput")
    bq = nc.dram_tensor("bq", [P, 2], F32, kind="ExternalInput")
    mmast = None
    maskt = None
    if mode == "causal":
        mmast = nc.dram_tensor("mmast", [P, P], F16, kind="ExternalInput")
    elif mode == "general":
        maskt = nc.dram_tensor("maskt", [NST, P, S], F16, kind="ExternalInput")
    # partials are gather-summed on the host; fp16 halves the output DMA
    out = nc.dram_tensor("out", [S, D], F16, kind="ExternalOutput")

    def nvalid_of(qc):
        return 4 * (qc + 1) if mode == "causal" else NST

    with TileContext(nc) as tc:
        with tc.tile_pool(name="big", bufs=1) as big:
            # warmup operands first on vector (gpsimd goes straight to DMA
            # issue).  Full K=128 matmuls: K=1 warmups don't register as
            # PE-busy for the HAM clock gate.
            warm = big.tile([1, QCW], F16, tag="warm", name="warm")
            wdum = big.tile([P, P], F16, tag="wdum", name="wdum")
            warm2 = big.tile([P, QCW], F16, tag="warm2", name="warm2")
            nc.vector.memset(wdum[:], 0.125)
            nc.vector.memset(warm2[:], 0.125)
            nc.vector.memset(warm[:], 1.0)
            # ---------- constants / biases ----------
            ones16 = big.tile([1, P], F16, tag="ones16", name="ones16")
            nc.vector.memset(ones16[:], 1.0)
            # ones row living at partition VW-1=64, matching the denominator
            # row of the attention psum for the broadcast matmul
            ones65 = big.tile([VW, HD], F16, tag="ones65", name="ones65")
            nc.vector.memset(ones65[HD:VW, :], 1.0)
            ebias = big.tile([P, 1], F32, tag="ebias", name="ebias")
            nc.vector.memset(ebias[:], EXP_BIAS)
            bq32 = big.tile([P, 2], F32, tag="bq32", name="bq32")
            mast16 = None
            if mode == "causal":
                mast16 = big.tile([P, P], F16, tag="mast16", name="mast16")

            # ---------- persistent fp16 operands ----------
            xT = big.tile([P, NQC, NKT, QCW], F16, tag="xT", name="xT")
            QT = [big.tile([P, S], F16, tag=f"QT{m}", name=f"QT{m}")
                  for m in range(2)]
            KT = [big.tile([P, S], F16, tag=f"KT{m}", name=f"KT{m}")
                  for m in range(2)]
            V = [big.tile([P, HC, VW], F16, tag=f"V{st}", name=f"V{st}")
                 for st in range(NST)]
            outT = [big.tile([P, S], F16, tag=f"outT{m}", name=f"outT{m}")
                    for m in range(2)]
            wq16 = big.tile([P, NKT, DC], F16, tag="wq16", name="wq16")
            wk16 = big.tile([P, NKT, DC], F16, tag="wk16", name="wk16")
            wv16 = big.tile([P, NKT, DC], F16, tag="wv16", name="wv16")
            wo16 = big.tile([P, 2, D], F16, tag="wo16", name="wo16")

            # ones columns of V are set once; v_chain only writes cols 0:HD
            for st in range(NST):
                nc.vector.memset(V[st][:, :, HD:VW], 1.0)

            with nc.named_scope("prep"):
                # load the Exp table while DMAs run
                wact = big.tile([1, QCW], F16, tag="wact", name="wact")
                nc.scalar.activation(wact[:], warm[:], AF.Exp,
                                     bias=ebias[0:1, :], scale=0.125)
                # Strict need-order across queues: qk_chain(0,0) wants
                # wq+wk+xT qc0 first; v_chains want wv right after; late x
                # chunks and wo are deferred so they don't steal bandwidth
                # from the critical first 2.5MB.
                nc.sync.dma_start(wq16[:], wq.ap())
                nc.scalar.dma_start(wk16[:], wk.ap())
                nc.gpsimd.dma_start(bq32[:], bq.ap())
                # kt-split keeps each descriptor a contiguous 4KB run
                nc.sync.dma_start(xT[:, 0, 0:4], xt_in[:, 0, 0:4])
                nc.scalar.dma_start(xT[:, 0, 4:8], xt_in[:, 0, 4:8])
                nc.gpsimd.dma_start(wv16[:], wv.ap())
                if mode == "causal":
                    nc.gpsimd.dma_start(mast16[:], mmast[:])
                nc.sync.dma_start(xT[:, 1], xt_in[:, 1])
                nc.scalar.dma_start(xT[:, 2], xt_in[:, 2])
                nc.gpsimd.dma_start(xT[:, 3], xt_in[:, 3])
                # wo isn't needed until the first d_chunk (~halfway in)
                nc.gpsimd.dma_start(wo16[:], wo.ap())

            # ---------- attention + interleaved projections ----------
            with nc.named_scope("main"), \
                 tc.tile_pool(name="sx", bufs=2, space="PSUM") as sxp, \
                 tc.tile_pool(name="oab", bufs=1, space="PSUM") as oabp, \
                 tc.tile_pool(name="pj", bufs=2, space="PSUM") as pjp, \
                 tc.tile_pool(name="pp16", bufs=4) as pp16, \
                 tc.tile_pool(name="nrm", bufs=2) as nrm, \
                 tc.tile_pool(name="ost", bufs=3) as ost, \
                 tc.tile_pool(name="mt", bufs=1) as mtp:

                def q_chain(mb, qc, warmups=0):
                    qs = slice(qc * QCW, (qc + 1) * QCW)
                    ps = pjp.tile([P, QCW], F32, tag="pj", name="pj")
                    for w in range(warmups):
                        nc.tensor.matmul(ps[:], wdum[:], warm2[:],
                                         start=True, stop=True)
                    for kt in range(NKT):
                        nc.tensor.matmul(
                            ps[:], wq16[:, kt, mb * P:(mb + 1) * P],
                            xT[:, qc, kt, :],
                            start=(kt == 0), stop=(kt == NKT - 1))
                    nc.vector.tensor_scalar_add(
                        QT[mb][:, qs], ps[:], bq32[:, mb:mb + 1])

                def k_chain(mb, qc):
                    # k-side bias is softmax-invariant: plain cast only
                    qs = slice(qc * QCW, (qc + 1) * QCW)
                    ps = pjp.tile([P, QCW], F32, tag="pj", name="pj")
                    for kt in range(NKT):
                        nc.tensor.matmul(
                            ps[:], wk16[:, kt, mb * P:(mb + 1) * P],
                            xT[:, qc, kt, :],
                            start=(kt == 0), stop=(kt == NKT - 1))
                    nc.vector.tensor_copy(KT[mb][:, qs], ps[:])

                def qk_chain(mb, qc, warmups=0):
                    q_chain(mb, qc, warmups)
                    k_chain(mb, qc)

                def v_chain(st):
                    ps = pjp.tile([P, QCW], F32, tag="pj", name="pj")
                    pv = ps[:, 0:DC]
                    sb = slice((st % 4) * P, (st % 4 + 1) * P)
                    for kt in range(NKT):
                        nc.tensor.matmul(
                            pv, xT[:, st // 4, kt, sb], wv16[:, kt, :],
                            start=(kt == 0), stop=(kt == NKT - 1))
                    nc.vector.tensor_copy(
                        V[st][:, :, 0:HD],
                        ps[:, 0:DC].rearrange("p (h d) -> p h d", h=HC))

                # head start: only what attention-hp0 qc0 needs.  The PE
                # clock ramps to full speed after ~10 back-to-back matmuls;
                # more warmups only delay real work behind the prep DMAs.
                with nc.named_scope("proj0"):
                    qk_chain(0, 0, warmups=14)
                    # ascending: attention-qc0 consumes V[0] first now that
                    # band tiles run in ascending kt order at the chunk end
                    for st in (0, 1, 2, 3):
                        v_chain(st)

                # filler queue: emitted between attention iterations.  Front
                # section pops 1/iteration (v-chains stay >=1 q-chunk ahead
                # of their consumers); the tail stretches into attn0's late
                # q-chunks, which otherwise leave the PE underfed while exp
                # paces the loop.  qk(1,0)/qk(1,1) are deferred into attn1:
                # with attn1's reversed qc order they are needed last.
                pending = []
                pending += [functools.partial(q_chain, 0, 1),
                            functools.partial(k_chain, 0, 1)]
                pending += [functools.partial(v_chain, st)
                            for st in (7, 6, 5, 4)]
                pending += [functools.partial(q_chain, 0, 2),
                            functools.partial(k_chain, 0, 2)]
                pending += [functools.partial(v_chain, st)
                            for st in (11, 10, 9, 8)]
                pending += [functools.partial(q_chain, 0, 3),
                            functools.partial(k_chain, 0, 3)]
                pending += [functools.partial(v_chain, st)
                            for st in (15, 14, 13, 12)]
                pending += [functools.partial(q_chain, 1, 2),
                            functools.partial(k_chain, 1, 2),
                            functools.partial(q_chain, 1, 3),
                            functools.partial(k_chain, 1, 3)]
                pending += [functools.partial(q_chain, 1, 0),
                            functools.partial(k_chain, 1, 0),
                            functools.partial(q_chain, 1, 1),
                            functools.partial(k_chain, 1, 1)]
                # attn0: 1/iter for the first 14 pops, then stretched so the
                # late q-chunks keep some PE filler work.  25 of the 26
                # entries pop during attn0 (incl. one per normalize); the
                # last (k(1,1)) is kept to cover attn1-qc3's normalize.
                attn0_pop_iters = set(range(14)) | {16, 18, 20, 22,
                                                    26, 30, 34}

                # plain d_chunks run inside attn1 where scalar is saturated
                # with exp: cast on vector -- except during the final flush,
                # where scalar is free and vector runs the reciprocal chain
                cast_on_scalar = [False]

                def d_chunk(qb, nh):
                    ns = slice(nh * QCW, (nh + 1) * QCW)
                    ps = pjp.tile([P, QCW], F32, tag="pj", name="pj")
                    for t in range(2):
                        nc.tensor.matmul(
                            ps[:], outT[t][:, qb * P:(qb + 1) * P],
                            wo16[:, t, ns], start=(t == 0), stop=(t == 1))
                    ob = ost.tile([P, QCW], F16, tag="ob", name="ob")
                    if cast_on_scalar[0]:
                        nc.scalar.copy(ob[:], ps[:])
                    else:
                        nc.vector.tensor_copy(ob[:], ps[:])
                    # keep output stores off the scalar queue: exp lives there
                    oeng = (nc.sync, nc.gpsimd)[(2 * qb + nh) % 2]
                    oeng.dma_start(out[qb * P:(qb + 1) * P, ns], ob[:])

                def d_chunk_wide(qb):
                    # final-flush variant: both Wo halves in one 2-bank psum
                    # (the sx pool is idle once attention is over), casts
                    # split across vector+scalar, single 256KB store
                    ps = sxp.tile([P, 2, QCW], F32, tag="sx", name="dw")
                    for nh in range(2):
                        for t in range(2):
                            nc.tensor.matmul(
                                ps[:, nh, :], outT[t][:, qb * P:(qb + 1) * P],
                                wo16[:, t, nh * QCW:(nh + 1) * QCW],
                                start=(t == 0), stop=(t == 1))
                    ob = ost.tile([P, 2, QCW], F16, tag="ob", name="ob")
                    nc.vector.tensor_copy(ob[:, 0, :], ps[:, 0, :])
                    nc.scalar.copy(ob[:, 1, :], ps[:, 1, :])
                    oeng = (nc.sync, nc.gpsimd)[qb % 2]
                    oeng.dma_start(out[qb * P:(qb + 1) * P, :],
                                   ob[:].rearrange("p a b -> p (a b)"))

                def qc_done(hp, qc, wide=False):
                    if hp == 1:
                        for qb in range(4 * qc, 4 * qc + 4):
                            if wide:
                                pending.append(
                                    functools.partial(d_chunk_wide, qb))
                            else:
                                pending.append(
                                    functools.partial(d_chunk, qb, 0))
                                pending.append(
                                    functools.partial(d_chunk, qb, 1))

                def pop_pending(n=1):
                    while n > 0 and pending:
                        pending.pop(0)()
                        n -= 1

                def attention(hp, interleave):
                    hA, hB = 2 * hp, 2 * hp + 1
                    maskt_sb = {}
                    if mode == "general":
                        for kt in range(NST):
                            mts = mtp.tile([P, S], F16, tag=f"mts{kt}",
                                           name=f"mts{kt}")
                            eng = (nc.sync, nc.scalar, nc.gpsimd)[kt % 3]
                            eng.dma_start(mts[:], maskt[kt])
                            maskt_sb[kt] = mts
                    def normalize(qc, oAB, last=False):
                        # One fp16 copy (*256 for fp16 range) frees the oAB
                        # psum bank after ~1.2us; the reciprocal chain runs
                        # off the SBUF copy, off the next q-chunk's critical
                        # path.  outT = (o*256) * 1/(den*256).
                        qs = slice(qc * QCW, (qc + 1) * QCW)
                        oc16 = nrm.tile([VW, 2 * QCW], F16, tag="oc16",
                                        name="oc16")
                        flush = last and hp == 1
                        with nc.allow_low_precision(
                                reason="softmax num/denom fp16 (scaled)"):
                            if flush:
                                # final chunk: every exp is done, so split
                                # the copy across scalar+vector to halve
                                # its latency
                                nc.scalar.mul(oc16[:, 0:QCW],
                                              oAB[0:VW, 0:QCW], 256.0)
                                nc.vector.tensor_scalar_mul(
                                    oc16[:, QCW:2 * QCW],
                                    oAB[0:VW, QCW:2 * QCW], 256.0)
                            elif hp == 1:
                                # scalar idles at attn1 chunk boundaries
                                # (the next chunk's exps aren't ready yet);
                                # vector is the backlogged engine there
                                nc.scalar.mul(oc16[:], oAB[0:VW, :], 256.0)
                            else:
                                nc.vector.tensor_scalar_mul(
                                    oc16[:], oAB[0:VW, :], 256.0)
                        # PE filler between the last PV and the bc matmuls:
                        # the in-order PE queue would otherwise stall on the
                        # oc16 copy at every q-chunk boundary
                        if interleave:
                            if flush:
                                cast_on_scalar[0] = True
                            pop_pending(2 if flush else 1)
                        for half, row in ((0, 0), (1, HD)):
                            hs = slice(half * QCW, (half + 1) * QCW)
                            bc_ps = pjp.tile([HD, QCW], F32, tag="pj",
                                             name="bc")
                            nc.tensor.matmul(bc_ps[:], ones65[HD:VW, :],
                                             oc16[HD:VW, hs],
                                             start=True, stop=True)
                            rdb = nrm.tile([HD, QCW], F32, tag="rdb",
                                           name="rdb")
                            nc.vector.reciprocal_approx_fast(rdb[:], bc_ps[:])
                            nc.vector.tensor_tensor(
                                outT[hp][row:row + HD, qs], oc16[0:HD, hs],
                                rdb[:], OP.mult)
                        if interleave:
                            if flush:
                                # remaining plain chunks fill the PE while
                                # the reciprocal chain runs on vector
                                pop_pending(len(pending))
                            qc_done(hp, qc, wide=flush)

                    qc_order = list(range(NQC))
                    if hp == 1 and mode == "causal":
                        qc_order = [3, 2, 1, 0]
                    it_global = 0
                    for qc in qc_order:
                        qs = slice(qc * QCW, (qc + 1) * QCW)
                        nvalid = nvalid_of(qc)
                        if mode == "causal":
                            # full tiles first (first write covers the whole
                            # bank), then the diagonal-band tiles narrowest
                            # last: their short exps drain the scalar backlog
                            # ahead of the q-chunk boundary
                            order = [(kt, None) for kt in range(4 * qc)]
                            order += [(4 * qc, 0), (4 * qc + 1, 1),
                                      (4 * qc + 2, 2), (4 * qc + 3, 3)]
                        else:
                            order = [(kt, None) for kt in range(nvalid)]
                        oAB = oabp.tile([P, 2 * QCW], F32, tag="oAB",
                                        name="oAB")

                        def emit_pv(prev, oAB=oAB):
                            kt_, w0_, p16_, st_, sp_ = prev
                            nc.tensor.matmul(
                                oAB[0:VW, w0_:QCW], V[kt_][:, hA, :],
                                p16_[:, 0, w0_:QCW], start=st_, stop=sp_)
                            nc.tensor.matmul(
                                oAB[0:VW, QCW + w0_:2 * QCW],
                                V[kt_][:, hB, :],
                                p16_[:, 1, w0_:QCW], start=st_, stop=sp_)

                        prev = None
                        for i, (kt, band_t) in enumerate(order):
                            ks = slice(kt * P, (kt + 1) * P)
                            w0 = 0 if band_t is None else P * band_t
                            # fillers first: PE work that overlaps the
                            # previous iteration's exp.  In attn1, pace at
                            # 1/iter and always keep one entry in reserve
                            # for the normalize at the chunk boundary.
                            if interleave and pending:
                                if hp == 0:
                                    if it_global in attn0_pop_iters:
                                        pop_pending(1)
                                elif len(pending) > 1 and i < nvalid - 1:
                                    pop_pending(1)
                            it_global += 1
                            sx = sxp.tile([P, 2, QCW], F32, tag="sx",
                                          name="sx")
                            # two K=64 matmuls on disjoint PE row groups run
                            # concurrently (tile_position row packing)
                            nc.tensor.matmul(
                                sx[:, 0, w0:QCW], KT[hp][0:HD, ks],
                                QT[hp][0:HD, qc * QCW + w0:(qc + 1) * QCW],
                                start=True, stop=True)
                            nc.tensor.matmul(
                                sx[:, 1, w0:QCW], KT[hp][HD:P, ks],
                                QT[hp][HD:P, qc * QCW + w0:(qc + 1) * QCW],
                                start=True, stop=True)
                            if prev is not None:
                                emit_pv(prev)
                            p16 = pp16.tile([P, 2, QCW], F16, tag="p16",
                                            name="p16")
                            nc.scalar.activation(p16[:, :, w0:QCW],
                                                 sx[:, :, w0:QCW], AF.Exp,
                                                 bias=ebias[:], scale=0.125)
                            if band_t is not None:
                                # triangular mask on the 128-wide diagonal
                                # block only; SBUF-only op -> GpSimd
                                blk = slice(w0, w0 + P)
                                nc.gpsimd.tensor_tensor(
                                    p16[:, :, blk], p16[:, :, blk],
                                    mast16[:].unsqueeze(1).to_broadcast(
                                        (P, 2, P)),
                                    OP.mult)
                            elif mode == "general":
                                nc.gpsimd.tensor_tensor(
                                    p16[:], p16[:],
                                    maskt_sb[kt][:, qs].unsqueeze(
                                        1).to_broadcast((P, 2, QCW)),
                                    OP.mult)
                            prev = (kt, w0, p16, i == 0, i == nvalid - 1)
                        emit_pv(prev)
                        normalize(qc, oAB, last=(qc == qc_order[-1]))
                    while interleave and pending:
                        pending.pop(0)()

                with nc.named_scope("attn0"):
                    attention(0, True)
                with nc.named_scope("attn1"):
                    attention(1, True)

    nc.compile()
    return nc


_BUILD_CACHE = {}


def _get_module(mode):
    if mode not in _BUILD_CACHE:
        _BUILD_CACHE[mode] = _build(mode)
    return _BUILD_CACHE[mode]


def _causal_master():
    kk = np.arange(P)[:, None]
    qq = np.arange(P)[None, :]
    return (kk <= qq).astype(np.float16)


def kernel(**inputs):
    x = np.ascontiguousarray(np.asarray(inputs["x"], dtype=np.float32))
    attn_mask = np.asarray(inputs["attn_mask"])
    Wq = np.asarray(inputs["Wq"], dtype=np.float32)
    Wk = np.asarray(inputs["Wk"], dtype=np.float32)
    Wv = np.asarray(inputs["Wv"], dtype=np.float32)
    Wo = np.asarray(inputs["Wo"], dtype=np.float32)
    bq = np.asarray(inputs["bq"], dtype=np.float32)
    bv = np.asarray(inputs["bv"], dtype=np.float32)
    bo = np.asarray(inputs["bo"], dtype=np.float32)

    m = attn_mask.reshape(B, attn_mask.shape[-2], attn_mask.shape[-1])
    if m.all():
        mode = "allones"
    elif all(np.array_equal(m[b], np.tril(np.ones((S, S), dtype=bool)))
             for b in range(B)):
        mode = "causal"
    else:
        mode = "general"

    nc = _get_module(mode)

    # device layouts are per-partition contiguous (see _build): [P, ...]
    xt_b = {}
    for b in range(B):
        xt = x[b].T.astype(np.float16).reshape(NKT, P, NQC, QCW)
        xt_b[b] = np.ascontiguousarray(xt.transpose(1, 2, 0, 3))

    def warr(w, cs):
        return np.ascontiguousarray(
            w[:, cs].astype(np.float16).reshape(NKT, P, DC).transpose(1, 0, 2))

    in_maps = []
    for c in range(NCORES):
        b, hg = c // HGROUPS, c % HGROUPS
        cs = slice(hg * DC, (hg + 1) * DC)
        im = {
            "xt": xt_b[b],
            "wq": warr(Wq, cs),
            "wk": warr(Wk, cs),
            "wv": warr(Wv, cs),
            "wo": np.ascontiguousarray(
                Wo[cs, :].astype(np.float16).reshape(2, P, D)
                .transpose(1, 0, 2)),
            "bq": np.ascontiguousarray(bq[cs].reshape(2, P).T),
        }
        if mode == "causal":
            im["mmast"] = _causal_master()
        elif mode == "general":
            im["maskt"] = np.ascontiguousarray(
                m[b].T.astype(np.float16).reshape(NST, P, S))
        in_maps.append(im)

    res = run_bass_kernel_spmd(nc, in_maps, core_ids=list(range(NCORES)))

    out = np.zeros((B, S, D), dtype=np.float32)
    for c in range(NCORES):
        out[c // HGROUPS] += res.results[c]["out"].astype(np.float32)
    # v-side bias: softmax weights sum to one, so attn(V+bv) = attn(V)+bv,
    # and (x+bv)@Wo folds into a constant row added with bo on the host
    out += (bo + bv @ Wo)[None, None, :]
    return out



# revision 19
# speedup vs baseline: 1.0081x; 1.0081x over previous
"""Multi-head attention (B=2, S=2048, D=1024, H=16) on 8 TRN2 NeuronCores.

Sharding: data-parallel over batch (2) x tensor-parallel over head groups (4).
Core c handles batch c//4, heads [4*(c%4), 4*(c%4)+4).  Each core computes its
heads' attention plus its slice of the output projection (Wo row-slice); the
final all-reduce over head groups happens during the host-side gather-sum.

Per-core device pipeline (matmul operands fp16, accumulation fp32):
  x^T staged in DRAM by the host -> Q^T/K^T ([128,2048] head-pair packed, head
  dims on partitions) and V ([2048, 4x65] with a ones column per head).
  scores^T = K @ Q^T per head via two row-tiled K=64 matmuls running
  concurrently on disjoint PE row groups -> exp on ScalarE (scale=1/8,
  bias=-12 for fp16 range) -> causal diagonal-block mask multiply on GpSimd
  -> PV with V_aug stationary, whose ones column also accumulates the softmax
  denominators -> normalize (denom*256, PE ones-broadcast,
  reciprocal_approx_fast) -> out^T.T @ Wo.
Causal structure is exploited at 128-column granularity: diagonal-band tiles
compute only their valid query range (the narrow tiles are issued first in
each accumulation group; a full-width tile closes it).  k-side bias is
softmax-invariant and dropped; the v-side bias is folded into the host-side
output bias (softmax weights sum to one).  Projection matmul chains are
interleaved between attention iterations to keep the PE dense.
"""

import functools

import numpy as np

import concourse.bass as bass
import concourse.mybir as mybir
from concourse import bacc
from concourse.tile import TileContext
from concourse.bass_utils import run_bass_kernel_spmd

P = 128
S = 2048
D = 1024
H = 16
HD = 64
B = 2
NCORES = 8
HGROUPS = 4
HC = H // HGROUPS          # 4 heads per core
DC = HC * HD               # 256-wide weight slice per core
NST = S // P               # 16 s-tiles (= k-tiles inside attention)
NKT = D // P               # 8 contraction tiles for the projections
QCW = 512
NQC = S // QCW             # 4 q-chunks
VW = HD + 1                # V block width incl. ones column

F32 = mybir.dt.float32
F16 = mybir.dt.float16
AF = mybir.ActivationFunctionType
OP = mybir.AluOpType
EXP_BIAS = -12.0           # keeps exp(q.k/8 - 12) inside fp16 range


def _build(mode):
    """mode: 'causal' | 'allones' | 'general'."""
    nc = bacc.Bacc("TRN2", debug=False, num_devices=NCORES,
                   num_swdge_queues=4)

    # All inputs are pre-arranged by the host into per-partition-contiguous
    # layouts so every load is 128 large descriptors (4-8KB) instead of
    # thousands of 512B gathers -- the input DMA is descriptor-bound.
    xt_in = nc.dram_tensor("xt", [P, NQC, NKT, QCW], F16,
                           kind="ExternalInput")
    wq = nc.dram_tensor("wq", [P, NKT, DC], F16, kind="ExternalInput")
    wk = nc.dram_tensor("wk", [P, NKT, DC], F16, kind="ExternalInput")
    wv = nc.dram_tensor("wv", [P, NKT, DC], F16, kind="ExternalInput")
    wo = nc.dram_tensor("wo", [P, 2, D], F16, kind="ExternalInput")
    bq = nc.dram_tensor("bq", [P, 2], F32, kind="ExternalInput")
    mmast = None
    maskt = None
    if mode == "causal":
        mmast = nc.dram_tensor("mmast", [P, P], F16, kind="ExternalInput")
    elif mode == "general":
        maskt = nc.dram_tensor("maskt", [NST, P, S], F16, kind="ExternalInput")
    # partials are gather-summed on the host; fp16 halves the output DMA
    out = nc.dram_tensor("out", [S, D], F16, kind="ExternalOutput")

    def nvalid_of(qc):
        return 4 * (qc + 1) if mode == "causal" else NST

    with TileContext(nc) as tc:
        with tc.tile_pool(name="big", bufs=1) as big:
            # warmup operands first on vector (gpsimd goes straight to DMA
            # issue).  Full K=128 matmuls: K=1 warmups don't register as
            # PE-busy for the HAM clock gate.
            warm = big.tile([1, QCW], F16, tag="warm", name="warm")
            wdum = big.tile([P, P], F16, tag="wdum", name="wdum")
            warm2 = big.tile([P, QCW], F16, tag="warm2", name="warm2")
            nc.vector.memset(wdum[:], 0.125)
            nc.vector.memset(warm2[:], 0.125)
            nc.vector.memset(warm[:], 1.0)
            # ---------- constants / biases ----------
            ones16 = big.tile([1, P], F16, tag="ones16", name="ones16")
            nc.vector.memset(ones16[:], 1.0)
            # ones row living at partition VW-1=64, matching the denominator
            # row of the attention psum for the broadcast matmul
            ones65 = big.tile([VW, HD], F16, tag="ones65", name="ones65")
            nc.vector.memset(ones65[HD:VW, :], 1.0)
            ebias = big.tile([P, 1], F32, tag="ebias", name="ebias")
            nc.vector.memset(ebias[:], EXP_BIAS)
            bq32 = big.tile([P, 2], F32, tag="bq32", name="bq32")
            mast16 = None
            if mode == "causal":
                mast16 = big.tile([P, P], F16, tag="mast16", name="mast16")

            # ---------- persistent fp16 operands ----------
            xT = big.tile([P, NQC, NKT, QCW], F16, tag="xT", name="xT")
            QT = [big.tile([P, S], F16, tag=f"QT{m}", name=f"QT{m}")
                  for m in range(2)]
            KT = [big.tile([P, S], F16, tag=f"KT{m}", name=f"KT{m}")
                  for m in range(2)]
            V = [big.tile([P, HC, VW], F16, tag=f"V{st}", name=f"V{st}")
                 for st in range(NST)]
            outT = [big.tile([P, S], F16, tag=f"outT{m}", name=f"outT{m}")
                    for m in range(2)]
            wq16 = big.tile([P, NKT, DC], F16, tag="wq16", name="wq16")
            wk16 = big.tile([P, NKT, DC], F16, tag="wk16", name="wk16")
            wv16 = big.tile([P, NKT, DC], F16, tag="wv16", name="wv16")
            wo16 = big.tile([P, 2, D], F16, tag="wo16", name="wo16")

            # ones columns of V are set once; v_chain only writes cols 0:HD
            for st in range(NST):
                nc.vector.memset(V[st][:, :, HD:VW], 1.0)

            with nc.named_scope("prep"):
                # load the Exp table while DMAs run
                wact = big.tile([1, QCW], F16, tag="wact", name="wact")
                nc.scalar.activation(wact[:], warm[:], AF.Exp,
                                     bias=ebias[0:1, :], scale=0.125)
                # Strict need-order across queues: qk_chain(0,0) wants
                # wq+wk+xT qc0 first; v_chains want wv right after; late x
                # chunks and wo are deferred so they don't steal bandwidth
                # from the critical first 2.5MB.
                # DMA exec is bandwidth-bound at the start, and the three
                # queues share the engine pool round-robin -- so every queue
                # must lead with first-needed data: wq+wk+xT qc0 (2MB for
                # qk_chain(0,0)), then wv, then the later x chunks, wo last.
                nc.sync.dma_start(wq16[:], wq.ap())
                nc.scalar.dma_start(wk16[:], wk.ap())
                nc.gpsimd.dma_start(bq32[:], bq.ap())
                # kt-split keeps each descriptor a contiguous 4KB run
                nc.gpsimd.dma_start(xT[:, 0, 0:3], xt_in[:, 0, 0:3])
                nc.sync.dma_start(xT[:, 0, 3:6], xt_in[:, 0, 3:6])
                nc.scalar.dma_start(xT[:, 0, 6:8], xt_in[:, 0, 6:8])
                nc.gpsimd.dma_start(wv16[:], wv.ap())
                if mode == "causal":
                    nc.scalar.dma_start(mast16[:], mmast[:])
                nc.sync.dma_start(xT[:, 1], xt_in[:, 1])
                nc.scalar.dma_start(xT[:, 2], xt_in[:, 2])
                nc.gpsimd.dma_start(xT[:, 3], xt_in[:, 3])
                # wo isn't needed until the first d_chunk (~halfway in)
                nc.sync.dma_start(wo16[:], wo.ap())

            # ---------- attention + interleaved projections ----------
            with nc.named_scope("main"), \
                 tc.tile_pool(name="sx", bufs=2, space="PSUM") as sxp, \
                 tc.tile_pool(name="oab", bufs=1, space="PSUM") as oabp, \
                 tc.tile_pool(name="pj", bufs=2, space="PSUM") as pjp, \
                 tc.tile_pool(name="pp16", bufs=4) as pp16, \
                 tc.tile_pool(name="nrm", bufs=2) as nrm, \
                 tc.tile_pool(name="ost", bufs=3) as ost, \
                 tc.tile_pool(name="mt", bufs=1) as mtp:

                def q_chain(mb, qc, warmups=0):
                    qs = slice(qc * QCW, (qc + 1) * QCW)
                    ps = pjp.tile([P, QCW], F32, tag="pj", name="pj")
                    for w in range(warmups):
                        nc.tensor.matmul(ps[:], wdum[:], warm2[:],
                                         start=True, stop=True)
                    for kt in range(NKT):
                        nc.tensor.matmul(
                            ps[:], wq16[:, kt, mb * P:(mb + 1) * P],
                            xT[:, qc, kt, :],
                            start=(kt == 0), stop=(kt == NKT - 1))
                    nc.vector.tensor_scalar_add(
                        QT[mb][:, qs], ps[:], bq32[:, mb:mb + 1])

                def k_chain(mb, qc):
                    # k-side bias is softmax-invariant: plain cast only
                    qs = slice(qc * QCW, (qc + 1) * QCW)
                    ps = pjp.tile([P, QCW], F32, tag="pj", name="pj")
                    for kt in range(NKT):
                        nc.tensor.matmul(
                            ps[:], wk16[:, kt, mb * P:(mb + 1) * P],
                            xT[:, qc, kt, :],
                            start=(kt == 0), stop=(kt == NKT - 1))
                    nc.vector.tensor_copy(KT[mb][:, qs], ps[:])

                def qk_chain(mb, qc, warmups=0):
                    q_chain(mb, qc, warmups)
                    k_chain(mb, qc)

                def v_chain(st):
                    ps = pjp.tile([P, QCW], F32, tag="pj", name="pj")
                    pv = ps[:, 0:DC]
                    sb = slice((st % 4) * P, (st % 4 + 1) * P)
                    for kt in range(NKT):
                        nc.tensor.matmul(
                            pv, xT[:, st // 4, kt, sb], wv16[:, kt, :],
                            start=(kt == 0), stop=(kt == NKT - 1))
                    nc.vector.tensor_copy(
                        V[st][:, :, 0:HD],
                        ps[:, 0:DC].rearrange("p (h d) -> p h d", h=HC))

                # head start: only what attention-hp0 qc0 needs.  The PE
                # clock ramps to full speed after ~10 back-to-back matmuls;
                # more warmups only delay real work behind the prep DMAs.
                with nc.named_scope("proj0"):
                    qk_chain(0, 0, warmups=14)
                    # ascending: attention-qc0 consumes V[0] first now that
                    # band tiles run in ascending kt order at the chunk end
                    for st in (0, 1, 2, 3):
                        v_chain(st)

                # filler queue: emitted between attention iterations.  Front
                # section pops 1/iteration (v-chains stay >=1 q-chunk ahead
                # of their consumers); the tail stretches into attn0's late
                # q-chunks, which otherwise leave the PE underfed while exp
                # paces the loop.  qk(1,0)/qk(1,1) are deferred into attn1:
                # with attn1's reversed qc order they are needed last.
                pending = []
                pending += [functools.partial(q_chain, 0, 1),
                            functools.partial(k_chain, 0, 1)]
                pending += [functools.partial(v_chain, st)
                            for st in (7, 6, 5, 4)]
                pending += [functools.partial(q_chain, 0, 2),
                            functools.partial(k_chain, 0, 2)]
                pending += [functools.partial(v_chain, st)
                            for st in (11, 10, 9, 8)]
                pending += [functools.partial(q_chain, 0, 3),
                            functools.partial(k_chain, 0, 3)]
                pending += [functools.partial(v_chain, st)
                            for st in (15, 14, 13, 12)]
                pending += [functools.partial(q_chain, 1, 2),
                            functools.partial(k_chain, 1, 2),
                            functools.partial(q_chain, 1, 3),
                            functools.partial(k_chain, 1, 3)]
                pending += [functools.partial(q_chain, 1, 0),
                            functools.partial(k_chain, 1, 0),
                            functools.partial(q_chain, 1, 1),
                            functools.partial(k_chain, 1, 1)]
                # attn0: 1/iter for the first 14 pops, then stretched so the
                # late q-chunks keep some PE filler work.  25 of the 26
                # entries pop during attn0 (incl. one per normalize); the
                # last (k(1,1)) is kept to cover attn1-qc3's normalize.
                attn0_pop_iters = set(range(14)) | {16, 18, 20, 22,
                                                    26, 30, 34}

                # plain d_chunks run inside attn1 where scalar is saturated
                # with exp: cast on vector -- except during the final flush,
                # where scalar is free and vector runs the reciprocal chain
                cast_on_scalar = [False]

                def d_chunk(qb, nh):
                    ns = slice(nh * QCW, (nh + 1) * QCW)
                    ps = pjp.tile([P, QCW], F32, tag="pj", name="pj")
                    for t in range(2):
                        nc.tensor.matmul(
                            ps[:], outT[t][:, qb * P:(qb + 1) * P],
                            wo16[:, t, ns], start=(t == 0), stop=(t == 1))
                    ob = ost.tile([P, QCW], F16, tag="ob", name="ob")
                    if cast_on_scalar[0]:
                        nc.scalar.copy(ob[:], ps[:])
                    else:
                        nc.vector.tensor_copy(ob[:], ps[:])
                    # keep output stores off the scalar queue: exp lives there
                    oeng = (nc.sync, nc.gpsimd)[(2 * qb + nh) % 2]
                    oeng.dma_start(out[qb * P:(qb + 1) * P, ns], ob[:])

                def d_chunk_wide(qb):
                    # final-flush variant: both Wo halves in one 2-bank psum
                    # (the sx pool is idle once attention is over), casts
                    # split across vector+scalar, single 256KB store
                    ps = sxp.tile([P, 2, QCW], F32, tag="sx", name="dw")
                    for nh in range(2):
                        for t in range(2):
                            nc.tensor.matmul(
                                ps[:, nh, :], outT[t][:, qb * P:(qb + 1) * P],
                                wo16[:, t, nh * QCW:(nh + 1) * QCW],
                                start=(t == 0), stop=(t == 1))
                    ob = ost.tile([P, 2, QCW], F16, tag="ob", name="ob")
                    nc.vector.tensor_copy(ob[:, 0, :], ps[:, 0, :])
                    nc.scalar.copy(ob[:, 1, :], ps[:, 1, :])
                    oeng = (nc.sync, nc.gpsimd)[qb % 2]
                    oeng.dma_start(out[qb * P:(qb + 1) * P, :],
                                   ob[:].rearrange("p a b -> p (a b)"))

                def qc_done(hp, qc, wide=False):
                    if hp == 1:
                        for qb in range(4 * qc, 4 * qc + 4):
                            if wide:
                                pending.append(
                                    functools.partial(d_chunk_wide, qb))
                            else:
                                pending.append(
                                    functools.partial(d_chunk, qb, 0))
                                pending.append(
                                    functools.partial(d_chunk, qb, 1))

                def pop_pending(n=1):
                    while n > 0 and pending:
                        pending.pop(0)()
                        n -= 1

                def attention(hp, interleave):
                    hA, hB = 2 * hp, 2 * hp + 1
                    maskt_sb = {}
                    if mode == "general":
                        for kt in range(NST):
                            mts = mtp.tile([P, S], F16, tag=f"mts{kt}",
                                           name=f"mts{kt}")
                            eng = (nc.sync, nc.scalar, nc.gpsimd)[kt % 3]
                            eng.dma_start(mts[:], maskt[kt])
                            maskt_sb[kt] = mts
                    def normalize(qc, oAB, last=False):
                        # One fp16 copy (*256 for fp16 range) frees the oAB
                        # psum bank after ~1.2us; the reciprocal chain runs
                        # off the SBUF copy, off the next q-chunk's critical
                        # path.  outT = (o*256) * 1/(den*256).
                        qs = slice(qc * QCW, (qc + 1) * QCW)
                        oc16 = nrm.tile([VW, 2 * QCW], F16, tag="oc16",
                                        name="oc16")
                        flush = last and hp == 1
                        with nc.allow_low_precision(
                                reason="softmax num/denom fp16 (scaled)"):
                            if flush:
                                # final chunk: every exp is done, so split
                                # the copy across scalar+vector to halve
                                # its latency
                                nc.scalar.mul(oc16[:, 0:QCW],
                                              oAB[0:VW, 0:QCW], 256.0)
                                nc.vector.tensor_scalar_mul(
                                    oc16[:, QCW:2 * QCW],
                                    oAB[0:VW, QCW:2 * QCW], 256.0)
                            elif hp == 1:
                                # scalar idles at attn1 chunk boundaries
                                # (the next chunk's exps aren't ready yet);
                                # vector is the backlogged engine there
                                nc.scalar.mul(oc16[:], oAB[0:VW, :], 256.0)
                            else:
                                nc.vector.tensor_scalar_mul(
                                    oc16[:], oAB[0:VW, :], 256.0)
                        # PE filler between the last PV and the bc matmuls:
                        # the in-order PE queue would otherwise stall on the
                        # oc16 copy at every q-chunk boundary
                        if interleave:
                            if flush:
                                cast_on_scalar[0] = True
                            pop_pending(2 if flush else 1)
                        for half, row in ((0, 0), (1, HD)):
                            hs = slice(half * QCW, (half + 1) * QCW)
                            bc_ps = pjp.tile([HD, QCW], F32, tag="pj",
                                             name="bc")
                            nc.tensor.matmul(bc_ps[:], ones65[HD:VW, :],
                                             oc16[HD:VW, hs],
                                             start=True, stop=True)
                            rdb = nrm.tile([HD, QCW], F32, tag="rdb",
                                           name="rdb")
                            nc.vector.reciprocal_approx_fast(rdb[:], bc_ps[:])
                            nc.vector.tensor_tensor(
                                outT[hp][row:row + HD, qs], oc16[0:HD, hs],
                                rdb[:], OP.mult)
                        if interleave:
                            if flush:
                                # remaining plain chunks fill the PE while
                                # the reciprocal chain runs on vector
                                pop_pending(len(pending))
                            qc_done(hp, qc, wide=flush)

                    qc_order = list(range(NQC))
                    if hp == 1 and mode == "causal":
                        qc_order = [3, 2, 1, 0]
                    it_global = 0
                    for qc in qc_order:
                        qs = slice(qc * QCW, (qc + 1) * QCW)
                        nvalid = nvalid_of(qc)
                        if mode == "causal":
                            # full tiles first (first write covers the whole
                            # bank), then the diagonal-band tiles narrowest
                            # last: their short exps drain the scalar backlog
                            # ahead of the q-chunk boundary
                            order = [(kt, None) for kt in range(4 * qc)]
                            order += [(4 * qc, 0), (4 * qc + 1, 1),
                                      (4 * qc + 2, 2), (4 * qc + 3, 3)]
                        else:
                            order = [(kt, None) for kt in range(nvalid)]
                        oAB = oabp.tile([P, 2 * QCW], F32, tag="oAB",
                                        name="oAB")

                        def emit_pv(prev, oAB=oAB):
                            kt_, w0_, p16_, st_, sp_ = prev
                            nc.tensor.matmul(
                                oAB[0:VW, w0_:QCW], V[kt_][:, hA, :],
                                p16_[:, 0, w0_:QCW], start=st_, stop=sp_)
                            nc.tensor.matmul(
                                oAB[0:VW, QCW + w0_:2 * QCW],
                                V[kt_][:, hB, :],
                                p16_[:, 1, w0_:QCW], start=st_, stop=sp_)

                        prev = None
                        for i, (kt, band_t) in enumerate(order):
                            ks = slice(kt * P, (kt + 1) * P)
                            w0 = 0 if band_t is None else P * band_t
                            # fillers first: PE work that overlaps the
                            # previous iteration's exp.  In attn1, pace at
                            # 1/iter and always keep one entry in reserve
                            # for the normalize at the chunk boundary.
                            if interleave and pending:
                                if hp == 0:
                                    if it_global in attn0_pop_iters:
                                        pop_pending(1)
                                elif len(pending) > 1 and i < nvalid - 1:
                                    pop_pending(1)
                            it_global += 1
                            sx = sxp.tile([P, 2, QCW], F32, tag="sx",
                                          name="sx")
                            # two K=64 matmuls on disjoint PE row groups run
                            # concurrently (tile_position row packing)
                            nc.tensor.matmul(
                                sx[:, 0, w0:QCW], KT[hp][0:HD, ks],
                                QT[hp][0:HD, qc * QCW + w0:(qc + 1) * QCW],
                                start=True, stop=True)
                            nc.tensor.matmul(
                                sx[:, 1, w0:QCW], KT[hp][HD:P, ks],
                                QT[hp][HD:P, qc * QCW + w0:(qc + 1) * QCW],
                                start=True, stop=True)
                            if prev is not None:
                                emit_pv(prev)
                            p16 = pp16.tile([P, 2, QCW], F16, tag="p16",
                                            name="p16")
                            nc.scalar.activation(p16[:, :, w0:QCW],
                                                 sx[:, :, w0:QCW], AF.Exp,
                                                 bias=ebias[:], scale=0.125)
                            if band_t is not None:
                                # triangular mask on the 128-wide diagonal
                                # block only; SBUF-only op -> GpSimd
                                blk = slice(w0, w0 + P)
                                nc.gpsimd.tensor_tensor(
                                    p16[:, :, blk], p16[:, :, blk],
                                    mast16[:].unsqueeze(1).to_broadcast(
                                        (P, 2, P)),
                                    OP.mult)
                            elif mode == "general":
                                nc.gpsimd.tensor_tensor(
                                    p16[:], p16[:],
                                    maskt_sb[kt][:, qs].unsqueeze(
                                        1).to_broadcast((P, 2, QCW)),
                                    OP.mult)
                            prev = (kt, w0, p16, i == 0, i == nvalid - 1)
                        emit_pv(prev)
                        normalize(qc, oAB, last=(qc == qc_order[-1]))
                    while interleave and pending:
                        pending.pop(0)()

                with nc.named_scope("attn0"):
                    attention(0, True)
                with nc.named_scope("attn1"):
                    attention(1, True)

    nc.compile()
    return nc


_BUILD_CACHE = {}


def _get_module(mode):
    if mode not in _BUILD_CACHE:
        _BUILD_CACHE[mode] = _build(mode)
    return _BUILD_CACHE[mode]


def _causal_master():
    kk = np.arange(P)[:, None]
    qq = np.arange(P)[None, :]
    return (kk <= qq).astype(np.float16)


def kernel(**inputs):
    x = np.ascontiguousarray(np.asarray(inputs["x"], dtype=np.float32))
    attn_mask = np.asarray(inputs["attn_mask"])
    Wq = np.asarray(inputs["Wq"], dtype=np.float32)
    Wk = np.asarray(inputs["Wk"], dtype=np.float32)
    Wv = np.asarray(inputs["Wv"], dtype=np.float32)
    Wo = np.asarray(inputs["Wo"], dtype=np.float32)
    bq = np.asarray(inputs["bq"], dtype=np.float32)
    bv = np.asarray(inputs["bv"], dtype=np.float32)
    bo = np.asarray(inputs["bo"], dtype=np.float32)

    m = attn_mask.reshape(B, attn_mask.shape[-2], attn_mask.shape[-1])
    if m.all():
        mode = "allones"
    elif all(np.array_equal(m[b], np.tril(np.ones((S, S), dtype=bool)))
             for b in range(B)):
        mode = "causal"
    else:
        mode = "general"

    nc = _get_module(mode)

    # device layouts are per-partition contiguous (see _build): [P, ...]
    xt_b = {}
    for b in range(B):
        xt = x[b].T.astype(np.float16).reshape(NKT, P, NQC, QCW)
        xt_b[b] = np.ascontiguousarray(xt.transpose(1, 2, 0, 3))

    def warr(w, cs):
        return np.ascontiguousarray(
            w[:, cs].astype(np.float16).reshape(NKT, P, DC).transpose(1, 0, 2))

    in_maps = []
    for c in range(NCORES):
        b, hg = c // HGROUPS, c % HGROUPS
        cs = slice(hg * DC, (hg + 1) * DC)
        im = {
            "xt": xt_b[b],
            "wq": warr(Wq, cs),
            "wk": warr(Wk, cs),
            "wv": warr(Wv, cs),
            "wo": np.ascontiguousarray(
                Wo[cs, :].astype(np.float16).reshape(2, P, D)
                .transpose(1, 0, 2)),
            "bq": np.ascontiguousarray(bq[cs].reshape(2, P).T),
        }
        if mode == "causal":
            im["mmast"] = _causal_master()
        elif mode == "general":
            im["maskt"] = np.ascontiguousarray(
                m[b].T.astype(np.float16).reshape(NST, P, S))
        in_maps.append(im)

    res = run_bass_kernel_spmd(nc, in_maps, core_ids=list(range(NCORES)))

    out = np.zeros((B, S, D), dtype=np.float32)
    for c in range(NCORES):
        out[c // HGROUPS] += res.results[c]["out"].astype(np.float32)
    # v-side bias: softmax weights sum to one, so attn(V+bv) = attn(V)+bv,
    # and (x+bv)@Wo folds into a constant row added with bo on the host
    out += (bo + bv @ Wo)[None, None, :]
    return out



# revision 26
# speedup vs baseline: 1.0133x; 1.0051x over previous
"""Multi-head attention (B=2, S=2048, D=1024, H=16) on 8 TRN2 NeuronCores.

Sharding: data-parallel over batch (2) x tensor-parallel over head groups (4).
Core c handles batch c//4, heads [4*(c%4), 4*(c%4)+4).  Each core computes its
heads' attention plus its slice of the output projection (Wo row-slice); the
final all-reduce over head groups happens during the host-side gather-sum.

Per-core device pipeline (matmul operands fp16, accumulation fp32):
  x^T staged in DRAM by the host -> Q^T/K^T ([128,2048] head-pair packed, head
  dims on partitions) and V ([2048, 4x65] with a ones column per head).
  scores^T = K @ Q^T per head via two row-tiled K=64 matmuls running
  concurrently on disjoint PE row groups -> exp on ScalarE (scale=1/8,
  bias=-12 for fp16 range) -> causal diagonal-block mask multiply on GpSimd
  -> PV with V_aug stationary, whose ones column also accumulates the softmax
  denominators -> normalize (denom*256, PE ones-broadcast,
  reciprocal_approx_fast) -> out^T.T @ Wo.
Causal structure is exploited at 128-column granularity: diagonal-band tiles
compute only their valid query range (the narrow tiles are issued first in
each accumulation group; a full-width tile closes it).  k-side bias is
softmax-invariant and dropped; the v-side bias is folded into the host-side
output bias (softmax weights sum to one).  Projection matmul chains are
interleaved between attention iterations to keep the PE dense.
"""

import functools

import numpy as np

import concourse.bass as bass
import concourse.mybir as mybir
from concourse import bacc
from concourse.tile import TileContext
from concourse.bass_utils import run_bass_kernel_spmd

P = 128
S = 2048
D = 1024
H = 16
HD = 64
B = 2
NCORES = 8
HGROUPS = 4
HC = H // HGROUPS          # 4 heads per core
DC = HC * HD               # 256-wide weight slice per core
NST = S // P               # 16 s-tiles (= k-tiles inside attention)
NKT = D // P               # 8 contraction tiles for the projections
QCW = 512
NQC = S // QCW             # 4 q-chunks
VW = HD + 1                # V block width incl. ones column

F32 = mybir.dt.float32
F16 = mybir.dt.float16
AF = mybir.ActivationFunctionType
OP = mybir.AluOpType
EXP_BIAS = -12.0           # keeps exp(q.k/8 - 12) inside fp16 range


def _build(mode):
    """mode: 'causal' | 'allones' | 'general'."""
    nc = bacc.Bacc("TRN2", debug=False, num_devices=NCORES,
                   num_swdge_queues=4)

    # All inputs are pre-arranged by the host into per-partition-contiguous
    # layouts so every load is 128 large descriptors (4-8KB) instead of
    # thousands of 512B gathers -- the input DMA is descriptor-bound.
    xt_in = nc.dram_tensor("xt", [P, NQC, NKT, QCW], F16,
                           kind="ExternalInput")
    wq = nc.dram_tensor("wq", [P, NKT, DC], F16, kind="ExternalInput")
    wk = nc.dram_tensor("wk", [P, NKT, DC], F16, kind="ExternalInput")
    wv = nc.dram_tensor("wv", [P, NKT, DC], F16, kind="ExternalInput")
    wo = nc.dram_tensor("wo", [P, 2, D], F16, kind="ExternalInput")
    bq = nc.dram_tensor("bq", [P, 2], F32, kind="ExternalInput")
    mmast = None
    maskt = None
    if mode == "causal":
        mmast = nc.dram_tensor("mmast", [P, P], F16, kind="ExternalInput")
    elif mode == "general":
        maskt = nc.dram_tensor("maskt", [NST, P, S], F16, kind="ExternalInput")
    # partials are gather-summed on the host; fp16 halves the output DMA
    out = nc.dram_tensor("out", [S, D], F16, kind="ExternalOutput")

    def nvalid_of(qc):
        return 4 * (qc + 1) if mode == "causal" else NST

    with TileContext(nc) as tc:
        with tc.tile_pool(name="big", bufs=1) as big:
            # warmup operands first on vector (gpsimd goes straight to DMA
            # issue).  Full K=128 matmuls: K=1 warmups don't register as
            # PE-busy for the HAM clock gate.
            warm = big.tile([1, QCW], F16, tag="warm", name="warm")
            wdum = big.tile([P, P], F16, tag="wdum", name="wdum")
            warm2 = big.tile([P, QCW], F16, tag="warm2", name="warm2")
            nc.vector.memset(wdum[:], 0.125)
            nc.vector.memset(warm2[:], 0.125)
            nc.vector.memset(warm[:], 1.0)
            # ---------- constants / biases ----------
            ones16 = big.tile([1, P], F16, tag="ones16", name="ones16")
            nc.vector.memset(ones16[:], 1.0)
            # ones row at partition 0 for the den-row broadcast matmuls
            ones_p0 = big.tile([1, HD], F16, tag="ones_p0", name="ones_p0")
            nc.vector.memset(ones_p0[:], 1.0)
            # ones row living at partition VW-1=64, matching the denominator
            # row of the attention psum for the broadcast matmul
            ones65 = big.tile([VW, HD], F16, tag="ones65", name="ones65")
            nc.vector.memset(ones65[HD:VW, :], 1.0)
            ebias = big.tile([P, 1], F32, tag="ebias", name="ebias")
            nc.vector.memset(ebias[:], EXP_BIAS)
            bq32 = big.tile([P, 2], F32, tag="bq32", name="bq32")
            mast16 = None
            if mode == "causal":
                mast16 = big.tile([P, P], F16, tag="mast16", name="mast16")

            # ---------- persistent fp16 operands ----------
            xT = big.tile([P, NQC, NKT, QCW], F16, tag="xT", name="xT")
            QT = [big.tile([P, S], F16, tag=f"QT{m}", name=f"QT{m}")
                  for m in range(2)]
            KT = [big.tile([P, S], F16, tag=f"KT{m}", name=f"KT{m}")
                  for m in range(2)]
            V = [big.tile([P, HC, VW], F16, tag=f"V{st}", name=f"V{st}")
                 for st in range(NST)]
            outT = [big.tile([P, S], F16, tag=f"outT{m}", name=f"outT{m}")
                    for m in range(2)]
            wq16 = big.tile([P, NKT, DC], F16, tag="wq16", name="wq16")
            wk16 = big.tile([P, NKT, DC], F16, tag="wk16", name="wk16")
            wv16 = big.tile([P, NKT, DC], F16, tag="wv16", name="wv16")
            wo16 = big.tile([P, 2, D], F16, tag="wo16", name="wo16")

            # ones columns of V are set once; v_chain only writes cols 0:HD
            for st in range(NST):
                nc.vector.memset(V[st][:, :, HD:VW], 1.0)

            with nc.named_scope("prep"):
                # load the Exp table while DMAs run
                wact = big.tile([1, QCW], F16, tag="wact", name="wact")
                nc.scalar.activation(wact[:], warm[:], AF.Exp,
                                     bias=ebias[0:1, :], scale=0.125)
                # Strict need-order across queues: qk_chain(0,0) wants
                # wq+wk+xT qc0 first; v_chains want wv right after; late x
                # chunks and wo are deferred so they don't steal bandwidth
                # from the critical first 2.5MB.
                # DMA exec is bandwidth-bound at the start, and the three
                # queues share the engine pool round-robin -- so every queue
                # must lead with first-needed data: wq+wk+xT qc0 (2MB for
                # qk_chain(0,0)), then wv, then the later x chunks, wo last.
                nc.sync.dma_start(wq16[:], wq.ap())
                nc.scalar.dma_start(wk16[:], wk.ap())
                nc.gpsimd.dma_start(bq32[:], bq.ap())
                # kt-split keeps each descriptor a contiguous 4KB run
                nc.gpsimd.dma_start(xT[:, 0, 0:3], xt_in[:, 0, 0:3])
                nc.sync.dma_start(xT[:, 0, 3:6], xt_in[:, 0, 3:6])
                nc.scalar.dma_start(xT[:, 0, 6:8], xt_in[:, 0, 6:8])
                nc.gpsimd.dma_start(wv16[:], wv.ap())
                if mode == "causal":
                    nc.scalar.dma_start(mast16[:], mmast[:])
                nc.sync.dma_start(xT[:, 1], xt_in[:, 1])
                nc.scalar.dma_start(xT[:, 2], xt_in[:, 2])
                nc.gpsimd.dma_start(xT[:, 3], xt_in[:, 3])
                # wo isn't needed until the first d_chunk (~halfway in)
                nc.sync.dma_start(wo16[:], wo.ap())

            # ---------- attention + interleaved projections ----------
            with nc.named_scope("main"), \
                 tc.tile_pool(name="sx", bufs=2, space="PSUM") as sxp, \
                 tc.tile_pool(name="oab", bufs=1, space="PSUM") as oabp, \
                 tc.tile_pool(name="pj", bufs=2, space="PSUM") as pjp, \
                 tc.tile_pool(name="pp16", bufs=4) as pp16, \
                 tc.tile_pool(name="nrm", bufs=2) as nrm, \
                 tc.tile_pool(name="ost", bufs=3) as ost, \
                 tc.tile_pool(name="mt", bufs=1) as mtp:

                def q_chain(mb, qc, warmups=0):
                    qs = slice(qc * QCW, (qc + 1) * QCW)
                    ps = pjp.tile([P, QCW], F32, tag="pj", name="pj")
                    for w in range(warmups):
                        nc.tensor.matmul(ps[:], wdum[:], warm2[:],
                                         start=True, stop=True)
                    for kt in range(NKT):
                        nc.tensor.matmul(
                            ps[:], wq16[:, kt, mb * P:(mb + 1) * P],
                            xT[:, qc, kt, :],
                            start=(kt == 0), stop=(kt == NKT - 1))
                    nc.vector.tensor_scalar_add(
                        QT[mb][:, qs], ps[:], bq32[:, mb:mb + 1])

                def k_chain(mb, qc):
                    # k-side bias is softmax-invariant: plain cast only
                    qs = slice(qc * QCW, (qc + 1) * QCW)
                    ps = pjp.tile([P, QCW], F32, tag="pj", name="pj")
                    for kt in range(NKT):
                        nc.tensor.matmul(
                            ps[:], wk16[:, kt, mb * P:(mb + 1) * P],
                            xT[:, qc, kt, :],
                            start=(kt == 0), stop=(kt == NKT - 1))
                    nc.vector.tensor_copy(KT[mb][:, qs], ps[:])

                def qk_chain(mb, qc, warmups=0):
                    q_chain(mb, qc, warmups)
                    k_chain(mb, qc)

                def v_chain(st):
                    ps = pjp.tile([P, QCW], F32, tag="pj", name="pj")
                    pv = ps[:, 0:DC]
                    sb = slice((st % 4) * P, (st % 4 + 1) * P)
                    for kt in range(NKT):
                        nc.tensor.matmul(
                            pv, xT[:, st // 4, kt, sb], wv16[:, kt, :],
                            start=(kt == 0), stop=(kt == NKT - 1))
                    nc.vector.tensor_copy(
                        V[st][:, :, 0:HD],
                        ps[:, 0:DC].rearrange("p (h d) -> p h d", h=HC))

                # head start, ordered to match DMA arrival: the PE clock
                # ramps over ~10 warmups while the critical 2MB loads, then
                # qk(0,0) / v0-3 / qk(0,1) consume tensors in the order the
                # bandwidth-bound prep stream delivers them.
                with nc.named_scope("proj0"):
                    qk_chain(0, 0, warmups=11)
                    # ascending: attention-qc0 consumes V[0] first now that
                    # band tiles run in ascending kt order at the chunk end
                    for st in (0, 1, 2, 3):
                        v_chain(st)
                    qk_chain(0, 1)

                # filler queue: emitted between attention iterations.  Front
                # section pops 1/iteration (v-chains stay >=1 q-chunk ahead
                # of their consumers); the tail stretches into attn0's late
                # q-chunks, which otherwise leave the PE underfed while exp
                # paces the loop.  qk(1,0)/qk(1,1) are deferred into attn1:
                # with attn1's reversed qc order they are needed last.
                pending = []
                pending += [functools.partial(v_chain, st)
                            for st in (7, 6, 5, 4)]
                pending += [functools.partial(q_chain, 0, 2),
                            functools.partial(k_chain, 0, 2)]
                pending += [functools.partial(v_chain, st)
                            for st in (11, 10, 9, 8)]
                pending += [functools.partial(q_chain, 0, 3),
                            functools.partial(k_chain, 0, 3)]
                pending += [functools.partial(v_chain, st)
                            for st in (15, 14, 13, 12)]
                pending += [functools.partial(q_chain, 1, 2),
                            functools.partial(k_chain, 1, 2),
                            functools.partial(q_chain, 1, 3),
                            functools.partial(k_chain, 1, 3)]
                pending += [functools.partial(q_chain, 1, 0),
                            functools.partial(k_chain, 1, 0),
                            functools.partial(q_chain, 1, 1),
                            functools.partial(k_chain, 1, 1)]
                # attn0: 1/iter until the front section drains, then spaced
                # over the late q-chunks (which otherwise leave the PE
                # underfed while exp paces the loop)
                attn0_pop_iters = set(range(16)) | {17, 19, 21, 23,
                                                    26, 29, 32}

                # plain d_chunks run inside attn1 where scalar is saturated
                # with exp: cast on vector -- except during the final flush,
                # where scalar is free and vector runs the reciprocal chain
                cast_on_scalar = [False]

                def d_chunk(qb, nh):
                    ns = slice(nh * QCW, (nh + 1) * QCW)
                    ps = pjp.tile([P, QCW], F32, tag="pj", name="pj")
                    for t in range(2):
                        nc.tensor.matmul(
                            ps[:], outT[t][:, qb * P:(qb + 1) * P],
                            wo16[:, t, ns], start=(t == 0), stop=(t == 1))
                    ob = ost.tile([P, QCW], F16, tag="ob", name="ob")
                    if cast_on_scalar[0]:
                        nc.scalar.copy(ob[:], ps[:])
                    else:
                        nc.vector.tensor_copy(ob[:], ps[:])
                    # keep output stores off the scalar queue: exp lives there
                    oeng = (nc.sync, nc.gpsimd)[(2 * qb + nh) % 2]
                    oeng.dma_start(out[qb * P:(qb + 1) * P, ns], ob[:])

                def d_chunk_wide(qb):
                    # final-flush variant: both Wo halves in one 2-bank psum
                    # (the sx pool is idle once attention is over), casts
                    # split across vector+scalar, single 256KB store
                    ps = sxp.tile([P, 2, QCW], F32, tag="sx", name="dw")
                    for nh in range(2):
                        for t in range(2):
                            nc.tensor.matmul(
                                ps[:, nh, :], outT[t][:, qb * P:(qb + 1) * P],
                                wo16[:, t, nh * QCW:(nh + 1) * QCW],
                                start=(t == 0), stop=(t == 1))
                    ob = ost.tile([P, 2, QCW], F16, tag="ob", name="ob")
                    nc.vector.tensor_copy(ob[:, 0, :], ps[:, 0, :])
                    nc.scalar.copy(ob[:, 1, :], ps[:, 1, :])
                    oeng = (nc.sync, nc.gpsimd)[qb % 2]
                    oeng.dma_start(out[qb * P:(qb + 1) * P, :],
                                   ob[:].rearrange("p a b -> p (a b)"))

                def qc_done(hp, qc, pos):
                    # pos = position of qc in the processing order.  The mix
                    # of plain/wide chunks is tuned to the shrinking filler
                    # demand of the later (smaller) q-chunks and the flush.
                    if hp != 1:
                        return
                    qbs = list(range(4 * qc, 4 * qc + 4))
                    plain = [functools.partial(d_chunk, qb, nh)
                             for qb in qbs for nh in (0, 1)]
                    wide = [functools.partial(d_chunk_wide, qb)
                            for qb in qbs]
                    if pos == 0:
                        pending.extend(plain)
                    elif pos == 1:
                        pending.extend(plain[0:4] + wide[2:4])
                    elif pos == 2:
                        pending.extend(wide[0:2] + plain[4:8])
                    else:
                        pending.extend(wide)

                def pop_pending(n=1):
                    while n > 0 and pending:
                        pending.pop(0)()
                        n -= 1

                def keep_warm(n):
                    # dummy matmuls so the PE clock gate doesn't drop during
                    # a dependency-latency window (cold matmuls run at ~2x)
                    ps = pjp.tile([P, QCW], F32, tag="pj", name="pj")
                    for _ in range(n):
                        nc.tensor.matmul(ps[:], wdum[:], warm2[:],
                                         start=True, stop=True)

                def attention(hp, interleave):
                    hA, hB = 2 * hp, 2 * hp + 1
                    maskt_sb = {}
                    if mode == "general":
                        for kt in range(NST):
                            mts = mtp.tile([P, S], F16, tag=f"mts{kt}",
                                           name=f"mts{kt}")
                            eng = (nc.sync, nc.scalar, nc.gpsimd)[kt % 3]
                            eng.dma_start(mts[:], maskt[kt])
                            maskt_sb[kt] = mts
                    def normalize(qc, oAB, pos):
                        # The tiny den-row copy goes first so the bc
                        # broadcast matmuls unblock ~0.6us before the bulk
                        # numerator copy lands; that copy frees the oAB
                        # psum banks.  outT = (o*256) * 1/(den*256).
                        qs = slice(qc * QCW, (qc + 1) * QCW)
                        flush = pos == NQC - 1 and hp == 1
                        den16 = nrm.tile([1, 2 * QCW], F16, tag="den16",
                                         name="den16")
                        num16 = nrm.tile([HD, 2 * QCW], F16, tag="oc16",
                                         name="oc16")
                        with nc.allow_low_precision(
                                reason="softmax num/denom fp16 (scaled)"):
                            nc.vector.tensor_scalar_mul(
                                den16[:], oAB[HD:VW, :], 256.0)
                            if hp == 1:
                                # scalar idles at attn1 chunk boundaries
                                # (the next chunk's exps aren't ready yet);
                                # vector is the backlogged engine there
                                nc.scalar.mul(num16[:], oAB[0:HD, :], 256.0)
                            else:
                                nc.vector.tensor_scalar_mul(
                                    num16[:], oAB[0:HD, :], 256.0)
                        # PE filler between the last PV and the bc matmuls:
                        # the in-order PE queue would otherwise stall on the
                        # den copy at every q-chunk boundary
                        if interleave:
                            if flush:
                                cast_on_scalar[0] = True
                            pop_pending(2 if flush else 1)
                        for half, row in ((0, 0), (1, HD)):
                            hs = slice(half * QCW, (half + 1) * QCW)
                            bc_ps = pjp.tile([HD, QCW], F32, tag="pj",
                                             name="bc")
                            nc.tensor.matmul(bc_ps[:], ones_p0[:],
                                             den16[0:1, hs],
                                             start=True, stop=True)
                            rdb = nrm.tile([HD, QCW], F32, tag="rdb",
                                           name="rdb")
                            nc.vector.reciprocal_approx_fast(rdb[:], bc_ps[:])
                            nc.vector.tensor_tensor(
                                outT[hp][row:row + HD, qs], num16[0:HD, hs],
                                rdb[:], OP.mult)
                        if interleave:
                            if flush:
                                # remaining chunks + keep-warm fill the PE
                                # while the reciprocal chain runs on vector
                                pop_pending(len(pending))
                                keep_warm(3)
                            qc_done(hp, qc, pos)

                    qc_order = list(range(NQC))
                    if hp == 1 and mode == "causal":
                        qc_order = [3, 2, 1, 0]
                    it_global = 0
                    for pos, qc in enumerate(qc_order):
                        qs = slice(qc * QCW, (qc + 1) * QCW)
                        nvalid = nvalid_of(qc)
                        if mode == "causal":
                            # full tiles first (first write covers the whole
                            # bank), then the diagonal-band tiles narrowest
                            # last: their short exps drain the scalar backlog
                            # ahead of the q-chunk boundary
                            order = [(kt, None) for kt in range(4 * qc)]
                            order += [(4 * qc, 0), (4 * qc + 1, 1),
                                      (4 * qc + 2, 2), (4 * qc + 3, 3)]
                        else:
                            order = [(kt, None) for kt in range(nvalid)]
                        oAB = oabp.tile([P, 2 * QCW], F32, tag="oAB",
                                        name="oAB")

                        def emit_pv(prev, oAB=oAB):
                            kt_, w0_, p16_, st_, sp_ = prev
                            nc.tensor.matmul(
                                oAB[0:VW, w0_:QCW], V[kt_][:, hA, :],
                                p16_[:, 0, w0_:QCW], start=st_, stop=sp_)
                            nc.tensor.matmul(
                                oAB[0:VW, QCW + w0_:2 * QCW],
                                V[kt_][:, hB, :],
                                p16_[:, 1, w0_:QCW], start=st_, stop=sp_)

                        prev = None
                        for i, (kt, band_t) in enumerate(order):
                            ks = slice(kt * P, (kt + 1) * P)
                            w0 = 0 if band_t is None else P * band_t
                            # fillers first: PE work that overlaps the
                            # previous iteration's exp.  In attn1, pace at
                            # 1/iter and always keep one entry in reserve
                            # for the normalize at the chunk boundary.
                            if interleave and pending:
                                if hp == 0:
                                    if it_global in attn0_pop_iters:
                                        pop_pending(1)
                                elif len(pending) > 1 and i < nvalid - 1:
                                    pop_pending(1)
                            it_global += 1
                            sx = sxp.tile([P, 2, QCW], F32, tag="sx",
                                          name="sx")
                            # two K=64 matmuls on disjoint PE row groups run
                            # concurrently (tile_position row packing)
                            nc.tensor.matmul(
                                sx[:, 0, w0:QCW], KT[hp][0:HD, ks],
                                QT[hp][0:HD, qc * QCW + w0:(qc + 1) * QCW],
                                start=True, stop=True)
                            nc.tensor.matmul(
                                sx[:, 1, w0:QCW], KT[hp][HD:P, ks],
                                QT[hp][HD:P, qc * QCW + w0:(qc + 1) * QCW],
                                start=True, stop=True)
                            if prev is not None:
                                emit_pv(prev)
                            p16 = pp16.tile([P, 2, QCW], F16, tag="p16",
                                            name="p16")
                            nc.scalar.activation(p16[:, :, w0:QCW],
                                                 sx[:, :, w0:QCW], AF.Exp,
                                                 bias=ebias[:], scale=0.125)
                            if band_t is not None:
                                # triangular mask on the 128-wide diagonal
                                # block only; SBUF-only op -> GpSimd
                                blk = slice(w0, w0 + P)
                                nc.gpsimd.tensor_tensor(
                                    p16[:, :, blk], p16[:, :, blk],
                                    mast16[:].unsqueeze(1).to_broadcast(
                                        (P, 2, P)),
                                    OP.mult)
                            elif mode == "general":
                                nc.gpsimd.tensor_tensor(
                                    p16[:], p16[:],
                                    maskt_sb[kt][:, qs].unsqueeze(
                                        1).to_broadcast((P, 2, QCW)),
                                    OP.mult)
                            prev = (kt, w0, p16, i == 0, i == nvalid - 1)
                        emit_pv(prev)
                        normalize(qc, oAB, pos)
                    while interleave and pending:
                        pending.pop(0)()

                with nc.named_scope("attn0"):
                    attention(0, True)
                with nc.named_scope("attn1"):
                    attention(1, True)

    nc.compile()
    return nc


_BUILD_CACHE = {}


def _get_module(mode):
    if mode not in _BUILD_CACHE:
        _BUILD_CACHE[mode] = _build(mode)
    return _BUILD_CACHE[mode]


def _causal_master():
    kk = np.arange(P)[:, None]
    qq = np.arange(P)[None, :]
    return (kk <= qq).astype(np.float16)


def kernel(**inputs):
    x = np.ascontiguousarray(np.asarray(inputs["x"], dtype=np.float32))
    attn_mask = np.asarray(inputs["attn_mask"])
    Wq = np.asarray(inputs["Wq"], dtype=np.float32)
    Wk = np.asarray(inputs["Wk"], dtype=np.float32)
    Wv = np.asarray(inputs["Wv"], dtype=np.float32)
    Wo = np.asarray(inputs["Wo"], dtype=np.float32)
    bq = np.asarray(inputs["bq"], dtype=np.float32)
    bv = np.asarray(inputs["bv"], dtype=np.float32)
    bo = np.asarray(inputs["bo"], dtype=np.float32)

    m = attn_mask.reshape(B, attn_mask.shape[-2], attn_mask.shape[-1])
    if m.all():
        mode = "allones"
    elif all(np.array_equal(m[b], np.tril(np.ones((S, S), dtype=bool)))
             for b in range(B)):
        mode = "causal"
    else:
        mode = "general"

    nc = _get_module(mode)

    # device layouts are per-partition contiguous (see _build): [P, ...]
    xt_b = {}
    for b in range(B):
        xt = x[b].T.astype(np.float16).reshape(NKT, P, NQC, QCW)
        xt_b[b] = np.ascontiguousarray(xt.transpose(1, 2, 0, 3))

    def warr(w, cs):
        return np.ascontiguousarray(
            w[:, cs].astype(np.float16).reshape(NKT, P, DC).transpose(1, 0, 2))

    in_maps = []
    for c in range(NCORES):
        b, hg = c // HGROUPS, c % HGROUPS
        cs = slice(hg * DC, (hg + 1) * DC)
        im = {
            "xt": xt_b[b],
            "wq": warr(Wq, cs),
            "wk": warr(Wk, cs),
            "wv": warr(Wv, cs),
            "wo": np.ascontiguousarray(
                Wo[cs, :].astype(np.float16).reshape(2, P, D)
                .transpose(1, 0, 2)),
            "bq": np.ascontiguousarray(bq[cs].reshape(2, P).T),
        }
        if mode == "causal":
            im["mmast"] = _causal_master()
        elif mode == "general":
            im["maskt"] = np.ascontiguousarray(
                m[b].T.astype(np.float16).reshape(NST, P, S))
        in_maps.append(im)

    res = run_bass_kernel_spmd(nc, in_maps, core_ids=list(range(NCORES)))

    out = np.zeros((B, S, D), dtype=np.float32)
    for c in range(NCORES):
        out[c // HGROUPS] += res.results[c]["out"].astype(np.float32)
    # v-side bias: softmax weights sum to one, so attn(V+bv) = attn(V)+bv,
    # and (x+bv)@Wo folds into a constant row added with bo on the host
    out += (bo + bv @ Wo)[None, None, :]
    return out

